# revision 1
# baseline (speedup 1.0000x reference)
"""Causal single-head attention (shared-weight multi-head), 8-core Trainium2 Bass kernel.

Problem: embedded [4, 4096, 1024] f32, Wq/Wk/Wv [1024, 64] f32.
  q/k/v = embedded @ W*;  S = q k^T / 8 (causal);  P = softmax(S);  head = P v
  output = tile(head, 16) -> [4, 4096, 1024] f32.

Sharding: 8 cores = 4 batches x 2 roles. Per batch the 4096 rows form 8 blocks
of 512; role 0 owns blocks {0,3,4,7}, role 1 owns {1,2,5,6} (causal work per
core is equal: sum(j+1) = 18 for both). Every core runs the SAME program over
4 "slots" (one per owned block, ascending); slot s processes a padded causal
extent of {1024,2048,3072,4096} columns. The host permutes each core's
transposed input so owned blocks sit first (positions 0-3) followed by the
other role's blocks (positions 4-7); causality then maps to a role-independent
set of column blocks per slot, with one potentially-padded column block per
slot whose validity is passed as per-core 0/1 data (padmask).

On-chip math (all matmuls bf16 with f32 PSUM accumulation):
  K^T [64, 4096] and Q^T [64, 2048] and V [4096, 64] projections
  S^T tile [128 cols, 512 rows] = (K^T chunk)^T-contract Q^T   (K=64 matmul)
  P~ = exp(S/8) via ACT (scores are bounded ~2.6, so no max subtraction),
       cast to bf16; causal tri-mask on the diagonal block, padmask on the
       padded block
  out [512 rows, 65] accumulated over col chunks with rhs = [V | 1]; column 64
       gives the softmax denominator for free. Divide via DVE reciprocal, then
       replicate x16 along features and DMA out.
"""

import os
import numpy as np
import ml_dtypes

B, T, E, HEAD, NH = 4, 4096, 1024, 64, 16
BLK = 512
NCORES = 8
OWN = {0: [0, 3, 4, 7], 1: [1, 2, 5, 6]}
PADS = [1, 2, 3, 4]  # padded count of "other role" 512-blocks visible per slot
PADMASK = {0: [0.0, 1.0, 0.0, 1.0], 1: [1.0, 0.0, 1.0, 0.0]}

_prog_cache = {}


def _build_program(reps=None):
    import concourse.bass as bass
    import concourse.mybir as mybir
    import concourse.tile as tile
    from concourse import bacc

    f32 = mybir.dt.float32
    bf16 = mybir.dt.bfloat16

    nc = bacc.Bacc("TRN2", target_bir_lowering=False, debug=False, num_devices=NCORES)

    xT = nc.dram_tensor("xT", [E, T], bf16, kind="ExternalInput").ap()
    wq = nc.dram_tensor("wq", [E, HEAD], bf16, kind="ExternalInput").ap()
    wk = nc.dram_tensor("wk", [E, HEAD], bf16, kind="ExternalInput").ap()
    wv = nc.dram_tensor("wv", [E, HEAD], bf16, kind="ExternalInput").ap()
    tri = nc.dram_tensor("tri", [128, 4, BLK], bf16, kind="ExternalInput").ap()
    padmask = nc.dram_tensor("padmask", [128, 4], f32, kind="ExternalInput").ap()
    out = nc.dram_tensor("out", [128, 16, HEAD], f32, kind="ExternalOutput").ap()

    KE = E // 128  # contraction chunks for projections

    import contextlib

    with tile.TileContext(nc) as tc:
        loop_ctx = tc.For_i(0, reps, 1) if reps else contextlib.nullcontext()
        with (
            loop_ctx,
            tc.tile_pool(name="singles", bufs=1) as singles,
            tc.tile_pool(name="psum_proj", bufs=2, space="PSUM") as psum_proj,
            tc.tile_pool(name="psum_s", bufs=2, space="PSUM") as psum_s,
            tc.tile_pool(name="psum_o", bufs=4, space="PSUM") as psum_o,
            tc.tile_pool(name="ptil", bufs=4) as ptil_pool,
            tc.tile_pool(name="work", bufs=4) as work,
        ):
            # ---- load inputs ----
            x_sb = singles.tile([128, KE, T], bf16)
            for k in range(KE):
                nc.sync.dma_start(out=x_sb[:, k, :], in_=xT[k * 128:(k + 1) * 128, :])
            wq_sb = singles.tile([128, KE, HEAD], bf16)
            wk_sb = singles.tile([128, KE, HEAD], bf16)
            wv_sb = singles.tile([128, KE, HEAD], bf16)
            nc.sync.dma_start(out=wq_sb, in_=wq.rearrange("(k p) d -> p k d", p=128))
            nc.sync.dma_start(out=wk_sb, in_=wk.rearrange("(k p) d -> p k d", p=128))
            nc.sync.dma_start(out=wv_sb, in_=wv.rearrange("(k p) d -> p k d", p=128))
            tri_sb = singles.tile([128, 4, BLK], bf16)
            nc.sync.dma_start(out=tri_sb, in_=tri)
            pm_sb = singles.tile([128, 4], f32)
            nc.sync.dma_start(out=pm_sb, in_=padmask)

            # ---- projections, per-block tiles emitted in slot-order waves
            # so attention slot s can start after wave s (fine-grained deps) ----
            KTb = [singles.tile([64, BLK], bf16, name=f"ktb{b}")
                   for b in range(8)]
            QTb = [singles.tile([64, BLK], bf16, name=f"qtb{b}")
                   for b in range(4)]
            V1b = [singles.tile([128, 4, HEAD + 1], bf16, name=f"v1b{b}")
                   for b in range(8)]

            def kproj(blk, wsb):
                ps = psum_proj.tile([64, BLK], f32, tag="proj", name=f"pk{blk}")
                for k in range(KE):
                    nc.tensor.matmul(
                        ps, wsb[:, k, :], x_sb[:, k, blk * BLK:(blk + 1) * BLK],
                        start=(k == 0), stop=(k == KE - 1),
                    )
                return ps

            def vproj(blk):
                for c in range(4):
                    g = blk * 4 + c
                    ps = psum_proj.tile([128, HEAD], f32, tag="proj",
                                        name=f"pv{g}")
                    for k in range(KE):
                        nc.tensor.matmul(
                            ps, x_sb[:, k, g * 128:(g + 1) * 128], wv_sb[:, k, :],
                            start=(k == 0), stop=(k == KE - 1),
                        )
                    nc.vector.tensor_copy(V1b[blk][:, c, 0:HEAD], ps)
                    nc.vector.memset(V1b[blk][:, c, HEAD:HEAD + 1], 1.0)

            outs_sb = singles.tile([128, 16, HEAD], f32)
            # ---- interleaved: wave w projections, then slot w attention
            # (slot s depends exactly on waves 0..s), so the PE priority
            # order alternates proj and scores and ACT fills early ----
            for s in range(4):
                bo, bx = s, 4 + s
                nc.vector.tensor_copy(KTb[bo], kproj(bo, wk_sb))
                nc.vector.tensor_copy(QTb[s], kproj(bo, wq_sb))
                nc.vector.tensor_copy(KTb[bx], kproj(bx, wk_sb))
                vproj(bo)
                vproj(bx)
                own_chunks = 4 * (s + 1)          # 128-col chunks in own region
                other_chunks = 4 * PADS[s]        # 128-col chunks in other region
                globs = list(range(own_chunks)) + [
                    16 + c for c in range(other_chunks)
                ]
                nC = len(globs)
                # one PSUM bank per 128-row chunk: accumulation groups must
                # not share a bank (start=True clears the bank zero-region)
                o_tiles = [psum_o.tile([128, HEAD + 1], f32, tag="o",
                                       name=f"o_s{s}r{r}")
                           for r in range(4)]
                for ci, g in enumerate(globs):
                    blk = g // 4 if g < 16 else 4 + (g - 16) // 4
                    sub = g % 4
                    s_ps = psum_s.tile([128, BLK], f32, tag="s")
                    nc.tensor.matmul(
                        s_ps, KTb[blk][:, sub * 128:(sub + 1) * 128],
                        QTb[s],
                        start=True, stop=True,
                    )
                    pt = ptil_pool.tile([128, BLK], bf16, tag="pt")
                    nc.scalar.activation(
                        pt, s_ps, mybir.ActivationFunctionType.Exp, scale=0.125
                    )
                    if ci >= 4 * s and ci < own_chunks:
                        nc.vector.tensor_mul(pt, pt, tri_sb[:, ci - 4 * s, :])
                    if ci >= own_chunks and (ci - own_chunks) // 4 == PADS[s] - 1:
                        nc.vector.tensor_scalar_mul(pt, pt, pm_sb[:, s:s + 1])
                    for r in range(4):
                        nc.tensor.matmul(
                            o_tiles[r], pt[:, r * 128:(r + 1) * 128],
                            V1b[blk][:, sub, :],
                            start=(ci == 0), stop=(ci == nC - 1),
                        )
                # ---- normalize into staging ----
                for r in range(4):
                    recip = work.tile([128, 1], f32, tag="recip")
                    nc.vector.reciprocal(recip, o_tiles[r][:, HEAD:HEAD + 1])
                    nc.vector.tensor_scalar_mul(
                        outs_sb[:, s * 4 + r, :], o_tiles[r][:, 0:HEAD], recip
                    )
            nc.sync.dma_start(out=out, in_=outs_sb)

    nc.compile()
    return nc


def _host_inputs(embedded, Wq, Wk, Wv):
    """Per-core input maps (host does layout only: transpose/permute/cast)."""
    bf = ml_dtypes.bfloat16
    emb = np.asarray(embedded, dtype=np.float32)
    wq = np.asarray(Wq, dtype=np.float32).astype(bf)
    wk = np.asarray(Wk, dtype=np.float32).astype(bf)
    wv = np.asarray(Wv, dtype=np.float32).astype(bf)

    # static triangular mask for the diagonal 512-block, [128, 4, 512]
    p = np.arange(128)[:, None, None]
    d = np.arange(4)[None, :, None]
    f = np.arange(BLK)[None, None, :]
    tri = ((d * 128 + p) <= f).astype(bf)

    in_maps = []
    for b in range(B):
        for role in range(2):
            order = OWN[role] + OWN[1 - role]
            xTb = emb[b].T  # [E, T]
            xTp = np.concatenate(
                [xTb[:, j * BLK:(j + 1) * BLK] for j in order], axis=1
            ).astype(bf)
            pm = np.broadcast_to(
                np.asarray(PADMASK[role], np.float32), (128, 4)
            ).astype(np.float32)
            in_maps.append({
                "xT": np.ascontiguousarray(xTp),
                "wq": wq, "wk": wk, "wv": wv,
                "tri": np.ascontiguousarray(tri),
                "padmask": np.ascontiguousarray(pm),
            })
    return in_maps


def _run(nc, in_maps, trace=False):
    from concourse.bass_utils import run_bass_kernel_spmd
    return run_bass_kernel_spmd(nc, in_maps, list(range(NCORES)), trace=trace)


def _assemble(results):
    head = np.empty((B, T, HEAD), dtype=np.float32)
    for core, r in enumerate(results):
        b, role = divmod(core, 2)
        o = np.asarray(r["out"])  # [128, 16, 64] partition-major
        o = o.transpose(1, 0, 2).reshape(16 * 128, HEAD)
        for s in range(4):
            j = OWN[role][s]
            head[b, j * BLK:(j + 1) * BLK, :] = o[s * BLK:(s + 1) * BLK, :]
    return np.tile(head, (1, 1, NH))


def kernel(embedded, Wq, Wk, Wv, num_heads):
    num_heads = int(num_heads)
    assert num_heads == NH

    if "nc" not in _prog_cache:
        _prog_cache["nc"] = _build_program()
    nc = _prog_cache["nc"]

    in_maps = _host_inputs(embedded, Wq, Wk, Wv)
    res = _run(nc, in_maps, trace=bool(int(os.environ.get("KERNEL_TRACE", "0"))))
    _prog_cache["last_result"] = res
    return _assemble(res.results)



# revision 9
# speedup vs baseline: 1.2306x; 1.2306x over previous
"""Causal single-head attention (shared-weight multi-head), 8-core Trainium2 Bass kernel.

Problem: embedded [4, 4096, 1024] f32, Wq/Wk/Wv [1024, 64] f32.
  q/k/v = embedded @ W*;  S = q k^T / 8 (causal);  P = softmax(S);  head = P v
  output = tile(head, 16) -> [4, 4096, 1024] f32.

Sharding: 8 cores = 4 batches x 2 parities. Core (b, r) owns q-rows
j*512 + 2*f + r for f in 0..255, j in 0..7 (every other row of every
512-block). Both cores of a batch then have IDENTICAL causal structure
(q-block j sees k-blocks 0..j) -- no padding, no role asymmetry; only the
triangular mask data differs per parity.

Per q-block j (8 per core): Q^T [64, 256] (own rows), K^T [64, 512] and
V [512, 65] per k-block (all rows; the 65th V column is ones so the PV
matmul accumulates the softmax denominator for free).
Scores are computed transposed, S^T quad = [128 kcols, 4, 256 q] in a
2-bank PSUM tile (4 matmuls, N=256); ONE activation instruction computes
exp(S/8) over the whole quad (cross-bank read) into bf16. PV accumulates
[128, 2, 65] into a single shared PSUM bank (one start=True zeroes the
bank; the second group accumulates onto zeros).

Projections use a fused [Wk|Wq] weight (M=128): K^T lands on partitions
0..63 (direct DVE copy), Q^T on 64..127 and hops to partitions 0..63 via
a small SBUF->SBUF DMA (stride-2 column select picks the parity rows).
Projection matmul chains for block j+1 are interleaved into attention(j)
to keep the PE busy while the ACT engine (the exp bottleneck) streams.
"""

import os
import numpy as np
import ml_dtypes

B, T, E, HEAD, NH = 4, 4096, 1024, 64, 16
BLK = 512
NB = T // BLK  # 8 q/k blocks
KE = E // 128  # contraction chunks
NCORES = 8

_prog_cache = {}


def _build_program(reps=None):
    import concourse.bass as bass
    import concourse.mybir as mybir
    import concourse.tile as tile
    from concourse import bacc

    f32 = mybir.dt.float32
    bf16 = mybir.dt.bfloat16

    nc = bacc.Bacc("TRN2", target_bir_lowering=False, debug=False, num_devices=NCORES)

    xT = nc.dram_tensor("xT", [E, T], bf16, kind="ExternalInput").ap()
    wkq = nc.dram_tensor("wkq", [E, 128], bf16, kind="ExternalInput").ap()
    wv = nc.dram_tensor("wv", [E, HEAD], bf16, kind="ExternalInput").ap()
    tri = nc.dram_tensor("tri", [128, 4, 256], bf16, kind="ExternalInput").ap()
    out = nc.dram_tensor("out", [128, 16, HEAD], f32, kind="ExternalOutput").ap()

    import contextlib

    with tile.TileContext(nc) as tc:
        loop_ctx = tc.For_i(0, reps, 1) if reps else contextlib.nullcontext()
        with (
            loop_ctx,
            tc.tile_pool(name="singles", bufs=1) as singles,
            tc.tile_pool(name="psum_proj", bufs=2, space="PSUM") as psum_proj,
            tc.tile_pool(name="psum_s", bufs=2, space="PSUM") as psum_s,
            tc.tile_pool(name="psum_o", bufs=2, space="PSUM") as psum_o,
            tc.tile_pool(name="ptil", bufs=3) as ptil_pool,
            tc.tile_pool(name="stage", bufs=2) as stage_pool,
            tc.tile_pool(name="work", bufs=4) as work,
        ):
            # ---- static inputs ----
            wkq_sb = singles.tile([128, KE, 128], bf16)
            wv_sb = singles.tile([128, KE, HEAD], bf16)
            tri_sb = singles.tile([128, 4, 256], bf16)
            nc.sync.dma_start(out=wkq_sb, in_=wkq.rearrange("(k p) d -> p k d", p=128))
            nc.sync.dma_start(out=wv_sb, in_=wv.rearrange("(k p) d -> p k d", p=128))
            nc.sync.dma_start(out=tri_sb, in_=tri)

            # ---- x, DMA'd per block in consumption order ----
            x_sb = singles.tile([128, KE, T], bf16)
            xr = xT.rearrange("(k p) t -> p k t", p=128)
            for j in range(NB):
                nc.sync.dma_start(
                    out=x_sb[:, :, j * BLK:(j + 1) * BLK],
                    in_=xr[:, :, j * BLK:(j + 1) * BLK],
                )

            kt_sb = singles.tile([64, NB, BLK], bf16)     # K^T per block
            qt_sb = singles.tile([64, NB, 256], bf16)     # Q^T per block (own rows)
            v1_sb = singles.tile([128, NB, 4, HEAD + 1], bf16)  # V | ones
            nc.vector.memset(v1_sb[:, :, :, HEAD:HEAD + 1], 1.0)
            outs_sb = singles.tile([128, 16, HEAD], f32)

            # ---------- projection emitters (as thunk lists) ----------
            def qk_chain_thunks(j):
                """[Wk|Wq] @ x_block -> psum [128, 512]; then copies + Q hop."""
                ps = psum_proj.tile([128, BLK], f32, tag="proj", name=f"pqk{j}")

                def mk_mm(k):
                    def t():
                        nc.tensor.matmul(
                            ps, wkq_sb[:, k, :],
                            x_sb[:, k, j * BLK:(j + 1) * BLK],
                            start=(k == 0), stop=(k == KE - 1),
                        )
                    return t

                def closer():
                    # K^T: partitions 0..63, straight copy
                    nc.vector.tensor_copy(kt_sb[:, j, :], ps[0:64, :])
                    # Q^T: partitions 64..127, parity-strided, hop to 0..63
                    st = stage_pool.tile([128, 256], bf16, tag="st")
                    nc.vector.tensor_copy(st[64:128, :], ps[64:128, 0:BLK:2])
                    nc.gpsimd.dma_start(out=qt_sb[:, j, :], in_=st[64:128, :])

                return [mk_mm(k) for k in range(KE)] + [closer]

            def v_chain_thunks(j):
                """x_block^T-stationary V projection -> one shared psum bank."""
                ps = psum_proj.tile([128, 4, HEAD], f32, tag="proj", name=f"pv{j}")

                def mk_mm(c, k):
                    def t():
                        nc.tensor.matmul(
                            ps[:, c, :],
                            x_sb[:, k, j * BLK + c * 128:j * BLK + (c + 1) * 128],
                            wv_sb[:, k, :],
                            start=(c == 0 and k == 0),
                            stop=(c == 3 and k == KE - 1),
                            skip_group_check=True,
                        )
                    return t

                def closer():
                    nc.vector.tensor_copy(v1_sb[:, j, :, 0:HEAD], ps)

                return [mk_mm(c, k) for c in range(4) for k in range(KE)] + [closer]

            # thunk queue: proj work to interleave into attention phases.
            # FIFO of (key, thunk); drain_key(k) emits everything queued up
            # to and including key k's thunks (dependency barrier).
            pending = []

            def pop_thunks(n):
                for _ in range(min(n, len(pending))):
                    pending.pop(0)[1]()

            def drain_key(key):
                while any(k == key for k, _ in pending):
                    pending.pop(0)[1]()

            # ---------- attention ----------
            def scores_quad(j, q):
                sq = psum_s.tile([128, 4, 256], f32, tag="s", name=f"s{j}_{q}")
                for c in range(4):
                    nc.tensor.matmul(
                        sq[:, c, :],
                        kt_sb[:, q, c * 128:(c + 1) * 128],
                        qt_sb[:, j, :],
                        start=(c % 2 == 0), stop=(c % 2 == 1),
                        skip_group_check=True,
                    )
                return sq

            def emit_phase(j):
                o_t = psum_o.tile([128, 2, HEAD + 1], f32, tag="o", name=f"o{j}")

                def pv(q, pt):
                    drain_key(("v", q))  # v1_sb[:, q] must be fully emitted
                    for c in range(4):
                        for g in range(2):
                            nc.tensor.matmul(
                                o_t[:, g, :],
                                pt[:, c, g * 128:(g + 1) * 128],
                                v1_sb[:, q, c, :],
                                start=(q == 0 and c == 0 and g == 0),
                                stop=(q == j and c == 3 and g == 1),
                                skip_group_check=True,
                            )

                drain_key(("qk", j))  # qt(j)/kt(j) must be fully emitted
                prev = None  # (q, pt) awaiting PV
                for q in range(j + 1):
                    sq = scores_quad(j, q)
                    pt = ptil_pool.tile([128, 4, 256], bf16, tag="pt")
                    nc.scalar.activation(
                        pt, sq, mybir.ActivationFunctionType.Exp, scale=0.125
                    )
                    if q == j:
                        nc.vector.tensor_mul(pt, pt, tri_sb)
                    if prev is not None:
                        pv(*prev)
                    prev = (q, pt)
                    # keep PE fed while ACT streams exp
                    pop_thunks(3 if j >= 2 else 12)
                pv(*prev)

                # normalize: divide by the ones-column accumulation
                recip = work.tile([128, 2, 1], f32, tag="recip")
                nc.vector.reciprocal(recip, o_t[:, :, HEAD:HEAD + 1])
                for g in range(2):
                    nc.vector.tensor_scalar_mul(
                        outs_sb[:, j * 2 + g, :], o_t[:, g, 0:HEAD], recip[:, g, :]
                    )
                nc.gpsimd.dma_start(
                    out=out[:, j * 2:j * 2 + 2, :],
                    in_=outs_sb[:, j * 2:j * 2 + 2, :],
                )

            # ---------- main loop ----------
            # Queue order per phase j: [qk(j+1), v(j)] -- qk(j+1) (incl. the
            # Q hop DMA) pops early so phase j+1's first scores quad never
            # waits on the hop; v(j) is only needed by phase j's diagonal PV
            # (barrier drains it there at the latest). Pacing pops interleave
            # the projection matmuls into the PE stream during attention.
            pending.extend((("qk", 0), t) for t in qk_chain_thunks(0))
            for j in range(NB):
                if j + 1 < NB:
                    pending.extend((("qk", j + 1), t) for t in qk_chain_thunks(j + 1))
                pending.extend((("v", j), t) for t in v_chain_thunks(j))
                emit_phase(j)
            pop_thunks(len(pending))

    nc.compile()
    return nc


def _host_inputs(embedded, Wq, Wk, Wv):
    """Per-core input maps (host does layout only: transpose/concat/cast)."""
    bf = ml_dtypes.bfloat16
    emb = np.asarray(embedded, dtype=np.float32)
    wkq = np.concatenate(
        [np.asarray(Wk, np.float32), np.asarray(Wq, np.float32)], axis=1
    ).astype(bf)
    wv = np.asarray(Wv, dtype=np.float32).astype(bf)

    # The program always takes the EVEN columns of its x as "own" q-rows, so
    # parity-1 cores get x^T with each adjacent column pair swapped; the true
    # global position of local k-column m is then m ^ r, which the causal
    # mask (applied on the diagonal quad only) accounts for.
    # tri[p, c, f] = 1 if ((c*128 + p) ^ r) <= (2*f + r) else 0
    p = np.arange(128)[:, None, None]
    c = np.arange(4)[None, :, None]
    f = np.arange(256)[None, None, :]
    tris = [(((c * 128 + p) ^ r) <= (2 * f + r)).astype(bf) for r in range(2)]

    swap = np.arange(T) ^ 1  # pair-swap permutation
    in_maps = []
    for b in range(B):
        xTb = emb[b].T.astype(bf)  # [E, T]
        xTs = [xTb, xTb[:, swap]]
        for r in range(2):
            in_maps.append({
                "xT": np.ascontiguousarray(xTs[r]),
                "wkq": wkq, "wv": wv,
                "tri": np.ascontiguousarray(tris[r]),
            })
    return in_maps


def _run(nc, in_maps, trace=False):
    from concourse.bass_utils import run_bass_kernel_spmd
    return run_bass_kernel_spmd(nc, in_maps, list(range(NCORES)), trace=trace)


def _assemble(results):
    head = np.empty((B, T, HEAD), dtype=np.float32)
    rows = np.arange(128)
    for core, res in enumerate(results):
        b, r = divmod(core, 2)
        o = np.asarray(res["out"])  # [128, 16, 64]
        for j in range(NB):
            for g in range(2):
                head[b, j * BLK + 2 * (g * 128 + rows) + r, :] = o[:, j * 2 + g, :]
    return np.tile(head, (1, 1, NH))


def kernel(embedded, Wq, Wk, Wv, num_heads):
    num_heads = int(num_heads)
    assert num_heads == NH

    if "nc" not in _prog_cache:
        _prog_cache["nc"] = _build_program()
    nc = _prog_cache["nc"]

    in_maps = _host_inputs(embedded, Wq, Wk, Wv)
    res = _run(nc, in_maps, trace=bool(int(os.environ.get("KERNEL_TRACE", "0"))))
    _prog_cache["last_result"] = res
    return _assemble(res.results)
